# revision 9
# baseline (speedup 1.0000x reference)
"""Trainium2 Bass kernel for nn_CrossAttentionModule_bias.

Math (B=2, C=256, H=W=64, N=4096):
    q = queries.reshape(B,C,N).T + q_pos        # [B,N,C]
    k = keys.reshape(B,C,N).T + k_pos
    v = values.reshape(B,C,N).T
    attn = softmax(q @ k.T / sqrt(C)) + c_b     # c_b: per-batch SCALAR
    out  = attn @ v   -> [B,C,H,W]

c_b = softplus(bias_eye*s_eye) + softplus(bias_mouth*s_mouth), a per-batch
scalar, so adding it to every attn entry is a rank-1 update:
    out[n,:] = (sum_m e[m,n] v[m,:]) / Z_n + c_b * S      (S = colsum V)
             = (sum_m e[m,n] (v[m,:] + c_b*S)) / Z_n
which folds ENTIRELY into V host-side (vfold = v + c_b*S per channel).

Device kernel (per core, 8 cores = 2 batches x 4 query shards, bf16):
    dotsT[m,n] = sum_c keff[c,m] qeff[c,n]      (keff stationary, qeff moving)
    e = exp(dotsT/16)  -> bf16                  (no max subtraction; |dots|<18)
    outT[n, 0:C] , Z[n] = sum_m e[m,n] vaug[m, 0:C+1]
        AV is TRANSPOSED vs the usual layout: e 128-col slices are the
        STATIONARY operand, vaug [128, C+2] (v | ones | pad) is the MOVING
        operand, so Z rides along as output column C -- no separate
        denominator matmul.  This makes the kernel exactly MAC-optimal:
        per body 2*N*NSH*(C + C/2 + 1) MACs = 131328 PE cycles ~= 54.7us.
    out[n,:] = outT[n, 0:C] * (1/Z[n])          (per-partition DVE scalar op)

Software pipeline: QK runs LOOKAHEAD steps ahead of AV so the ACT exp
latency (~720ns/tile) hides under PE work.  PSUM: 4 dots bufs + 4 outT
accumulators = 8 banks exactly.
"""

import numpy as np
import ml_dtypes

import concourse.bass as bass
import concourse.mybir as mybir
import concourse.tile as tile
from concourse import bacc
from concourse.bass_utils import run_bass_kernel_spmd

# Problem shape (hardcoded per the task contract)
B, C, H, W = 2, 256, 64, 64
N = H * W                      # 4096
NCORES = 8
SHARDS_PER_B = NCORES // B     # 4 query shards per batch
NSH = N // SHARDS_PER_B        # 1024 query columns per core
SCALE = float(C) ** -0.5       # 1/16
P = 128
CCN = C // P                   # 2 c-chunks
MCN = N // P                   # 32 m-chunks
NT_SIZE = 512                  # n-tile width (one PSUM bank of dots)
NTN = NSH // NT_SIZE           # 2 n-tiles per core
JN = NT_SIZE // P              # 4 outT row-chunks per n-tile
CP = C + 2                     # v columns | ones col | pad col
LOOKAHEAD = 3                  # QK tiles in flight ahead of AV

F32 = mybir.dt.float32
BF16 = mybir.dt.bfloat16

EXP = mybir.ActivationFunctionType.Exp

_CACHE: dict = {}


def _build_bass(reps: int = 1, loop_reps: int = 0, ablate: tuple = (), la: int = LOOKAHEAD):
    """reps>1 unrolls the whole compute; loop_reps>0 wraps it in a hardware
    For_i loop instead (timing-only variants: slope between two loop_reps
    builds isolates per-iteration HW time from the ~100ms dispatch floor)."""
    nc = bacc.Bacc("TRN2", target_bir_lowering=False, debug=False)

    keff = nc.dram_tensor("keff", [C, N], BF16, kind="ExternalInput")
    qeff = nc.dram_tensor("qeff", [C, NSH], BF16, kind="ExternalInput")
    vaug = nc.dram_tensor("vaug", [N, CP], BF16, kind="ExternalInput")
    out = nc.dram_tensor("out", [NSH, C], F32, kind="ExternalOutput")

    KQ = 8                     # m-chunks per keff DMA tile
    KQN = MCN // KQ            # 4 keff tiles per c-chunk

    with tile.TileContext(nc) as tc:
        with (
            tc.tile_pool(name="const", bufs=1) as cpool,
            tc.tile_pool(name="work", bufs=la + 1) as wpool,
            tc.tile_pool(name="tail", bufs=2) as tpool,
            tc.tile_pool(name="dots_ps", bufs=la + 1, space="PSUM") as dots_pool,
            tc.tile_pool(name="acc_ps", bufs=1, space="PSUM") as acc_pool,
        ):
            zero = cpool.tile([P, 1], F32, tag="zero", name="zero")
            nc.vector.memset(zero[:], 0.0)

            qeff_t = []
            for cc in range(CCN):
                t = cpool.tile([P, NSH], BF16, tag=f"qeff{cc}", name=f"qeff{cc}")
                nc.sync.dma_start(t[:], qeff[cc * P : (cc + 1) * P, :])
                qeff_t.append(t)

            # keff split into [128, KQ*128] tiles so QK can start early
            keff_t = [[None] * KQN for _ in range(CCN)]
            for q in range(KQN):
                for cc in range(CCN):
                    t = cpool.tile([P, KQ * P], BF16, tag=f"keff{cc}_{q}", name=f"keff{cc}_{q}")
                    nc.sync.dma_start(
                        t[:], keff[cc * P : (cc + 1) * P, q * KQ * P : (q + 1) * KQ * P]
                    )
                    keff_t[cc][q] = t

            vaug_t = []
            for mc in range(MCN):
                t = cpool.tile([P, CP], BF16, tag=f"vaug{mc}", name=f"vaug{mc}")
                nc.sync.dma_start(t[:], vaug[mc * P : (mc + 1) * P, :])
                vaug_t.append(t)

            const_expt = None
            if "exp" in ablate:
                const_expt = cpool.tile([P, NT_SIZE], BF16, tag="cexpt", name="cexpt")
                nc.vector.memset(const_expt[:], 1.0)

            qk1cc = "qk1cc" in ablate

            def emit_qk_cc(dots, nt, mc, cc):
                # One c-chunk contribution.  cc0 starts the group, cc1 stops
                # it; the two are emitted a pipeline step apart so adjacent
                # PE matmuls never target the same PSUM bank (back-to-back
                # same-bank matmuls serialize fill/drain: +~100ns/MM).
                ns = slice(nt * NT_SIZE, (nt + 1) * NT_SIZE)
                if "qksamew" in ablate:
                    lhsT = keff_t[0][0][:, 0:P]
                else:
                    lhsT = keff_t[cc][mc // KQ][:, (mc % KQ) * P : (mc % KQ + 1) * P]
                nc.tensor.matmul(
                    dots[:],
                    lhsT,
                    qeff_t[cc][:, ns],
                    start=(cc == 0),
                    stop=(cc == CCN - 1) or qk1cc,
                )

            def emit_qk(nt, mc):
                dots = dots_pool.tile([P, NT_SIZE], F32, tag="dots", name="dots")
                emit_qk_cc(dots, nt, mc, 0)
                if not qk1cc:
                    emit_qk_cc(dots, nt, mc, 1)
                return dots

            def emit_body():
                def alloc_ut():
                    return [
                        acc_pool.tile([P, CP], F32, tag=f"ut{j}", name=f"ut{j}")
                        for j in range(JN)
                    ]

                ut = alloc_ut()

                def emit_tail(nt):
                    # outT rows: out[n,:] = utj[n, 0:C] * (1/Z[n]); Z = col C
                    for j in range(JN):
                        r0 = nt * NT_SIZE + j * P
                        recip = tpool.tile([P, 1], F32, tag="recip", name="recip")
                        nc.vector.reciprocal(recip[:], ut[j][:, C : C + 1])
                        outsb = tpool.tile([P, C], F32, tag="outsb", name="outsb")
                        nc.vector.tensor_scalar_mul(outsb[:], ut[j][:, 0:C], recip[:])
                        nc.sync.dma_start(out[r0 : r0 + P, :], outsb[:])

                steps = [(nt, mc) for nt in range(NTN) for mc in range(MCN)]
                noqk = "noqk" in ablate
                noav = "noav" in ablate
                nsteps = len(steps)
                qk_q: list = []

                def qk0(s):
                    dots = dots_pool.tile([P, NT_SIZE], F32, tag="dots", name="dots")
                    emit_qk_cc(dots, *steps[s], 0)
                    qk_q.append(dots)

                def qk1(s):
                    if not qk1cc:
                        emit_qk_cc(qk_q[s - (len_done[0] )], *steps[s], 1)

                # staggered prologue: cc0(s) runs one step ahead of cc1(s)
                len_done = [0]  # index of qk_q[0] in step numbering
                if not noqk:
                    qk0(0)
                    for s in range(1, la):
                        qk0(s)
                        qk1(s - 1)
                for i, (nt, mc) in enumerate(steps):
                    if noqk:
                        # timing ablation: exp reads an unwritten PSUM tile
                        dots = dots_pool.tile([P, NT_SIZE], F32, tag="dots", name="dots")
                    else:
                        if i + la < nsteps:
                            qk0(i + la)
                        if i + la - 1 < nsteps:
                            qk1(i + la - 1)
                        dots = qk_q.pop(0)
                        len_done[0] += 1
                    if "exp" in ablate:
                        expt = const_expt
                    else:
                        expt = wpool.tile([P, NT_SIZE], BF16, tag="expt", name="expt")
                        nc.scalar.activation(expt[:], dots[:], EXP, bias=zero[:], scale=SCALE)
                    first, last = mc == 0, mc == MCN - 1
                    if not noav:
                        for j in range(JN):
                            lhsT = (
                                expt[:, 0:P]
                                if "avsamew" in ablate
                                else expt[:, j * P : (j + 1) * P]
                            )
                            nc.tensor.matmul(
                                ut[j][:],
                                lhsT,
                                vaug_t[0 if "avsamev" in ablate else mc][:],
                                start=first,
                                stop=last,
                            )
                    if last:
                        if "tail" not in ablate and not noav:
                            emit_tail(nt)
                        if nt + 1 < NTN:
                            ut = alloc_ut()

            if loop_reps > 0:
                with tc.For_i(0, loop_reps, 1, hint_engines=(mybir.EngineType.PE,)):
                    emit_body()
            else:
                for _ in range(reps):
                    emit_body()

    nc.compile()
    return nc


def _prep_inputs(queries, keys, values, mask_eye, mask_mouth, q_pos, k_pos,
                 bias_eye, bias_mouth):
    """Host-side shard prep: positional adds, V transpose + ones column, and
    the per-batch scalar bias folded into V (v + c_b * colsum(V))."""
    q = queries.reshape(B, C, N) + q_pos[0].T[None]
    k = keys.reshape(B, C, N) + k_pos[0].T[None]
    vT = np.ascontiguousarray(values.reshape(B, C, N).transpose(0, 2, 1))  # [B,N,C]

    def msum(mask):
        # nearest resize 128->64 picks every other row/col
        m = mask[:, :, ::2, ::2].reshape(B, -1)
        return (m * m).sum(axis=1, dtype=np.float64)

    softplus = lambda x: np.logaddexp(0.0, x)
    c_b = softplus(float(bias_eye[0]) * msum(mask_eye)) + softplus(
        float(bias_mouth[0]) * msum(mask_mouth)
    )  # [B]
    S = vT.sum(axis=1, dtype=np.float64)  # [B, C]

    vaug = np.zeros((B, N, CP), ml_dtypes.bfloat16)
    vaug[:, :, :C] = (vT + (c_b[:, None] * S)[:, None, :]).astype(ml_dtypes.bfloat16)
    vaug[:, :, C] = 1.0

    kb = k.astype(ml_dtypes.bfloat16)
    qb = q.astype(ml_dtypes.bfloat16)

    in_maps = []
    for core in range(NCORES):
        b, sh = divmod(core, SHARDS_PER_B)
        n0 = sh * NSH
        in_maps.append(
            {
                "keff": np.ascontiguousarray(kb[b]),
                "qeff": np.ascontiguousarray(qb[b][:, n0 : n0 + NSH]),
                "vaug": vaug[b],
            }
        )
    return in_maps


def kernel(**inputs) -> np.ndarray:
    inputs = {k: np.asarray(v, np.float32) for k, v in inputs.items()}
    in_maps = _prep_inputs(**inputs)

    if "nc" not in _CACHE:
        _CACHE["nc"] = _build_bass()
    res = run_bass_kernel_spmd(_CACHE["nc"], in_maps, list(range(NCORES)))

    full = np.empty((B, C, N), np.float32)
    for core in range(NCORES):
        b, sh = divmod(core, SHARDS_PER_B)
        n0 = sh * NSH
        full[b][:, n0 : n0 + NSH] = res.results[core]["out"].T
    return full.reshape(B, C, H, W)


# revision 14
# speedup vs baseline: 1.0302x; 1.0302x over previous
"""Trainium2 Bass kernel for nn_CrossAttentionModule_bias.

Math (B=2, C=256, H=W=64, N=4096):
    q = queries.reshape(B,C,N).T + q_pos        # [B,N,C]
    k = keys.reshape(B,C,N).T + k_pos
    v = values.reshape(B,C,N).T
    attn = softmax(q @ k.T / sqrt(C)) + c_b     # c_b: per-batch SCALAR
    out  = attn @ v   -> [B,C,H,W]

c_b = softplus(bias_eye*s_eye) + softplus(bias_mouth*s_mouth), a per-batch
scalar, so adding it to every attn entry is a rank-1 update:
    out[n,:] = (sum_m e[m,n] v[m,:]) / Z_n + c_b * S      (S = colsum V)
             = (sum_m e[m,n] (v[m,:] + c_b*S)) / Z_n
which folds ENTIRELY into V host-side (vfold = v + c_b*S per channel).

Device kernel (per core, 8 cores = 2 batches x 4 query shards, bf16):
    dotsT[m,n] = sum_c keff[c,m] qeff[c,n]      (keff stationary, qeff moving)
    e = exp(dotsT/16)  -> bf16                  (no max subtraction; |dots|<18)
    outT[n, 0:C] , Z[n] = sum_m e[m,n] vaug[m, 0:C+1]
        AV is TRANSPOSED vs the usual layout: e 128-col slices are the
        STATIONARY operand, vaug [128, C+2] (v | ones | pad) is the MOVING
        operand, so Z rides along as output column C -- no separate
        denominator matmul.  This makes the kernel exactly MAC-optimal:
        per body 2*N*NSH*(C + C/2 + 1) MACs = 131328 PE cycles ~= 54.7us.
    out[n,:] = outT[n, 0:C] * (1/Z[n])          (per-partition DVE scalar op)

HW cost model (measured): per matmul ~ moving_cols/2.4GHz + 4ns + ~80ns iff
the MOVING operand AP differs from the previous matmul's.  So the loop is
batched BATCH=4 m-chunks at a time: the 4 cc0 matmuls share one qeff AP
(1 switch), then the 4 cc1 matmuls (1 switch), then 16 AV matmuls switch
moving only once per m-chunk.  The 4 dots go to the 4 PSUM dots banks;
exp(batch) overlaps the next batch's QK on the PE.
PSUM: 4 dots banks + 4 outT accumulators = 8 exactly.
"""

import numpy as np
import ml_dtypes

import concourse.bass as bass
import concourse.mybir as mybir
import concourse.tile as tile
from concourse import bacc
from concourse.bass_utils import run_bass_kernel_spmd

# Problem shape (hardcoded per the task contract)
B, C, H, W = 2, 256, 64, 64
N = H * W                      # 4096
NCORES = 8
SHARDS_PER_B = NCORES // B     # 4 query shards per batch
NSH = N // SHARDS_PER_B        # 1024 query columns per core
SCALE = float(C) ** -0.5       # 1/16
P = 128
CCN = C // P                   # 2 c-chunks
MCN = N // P                   # 32 m-chunks
NT_SIZE = 512                  # n-tile width (one PSUM bank of dots)
NTN = NSH // NT_SIZE           # 2 n-tiles per core
JN = NT_SIZE // P              # 4 outT row-chunks per n-tile
CP = C + 2                     # v columns | ones col | pad col
BATCH = 4                      # m-chunks per pipeline batch (= dots banks)

F32 = mybir.dt.float32
BF16 = mybir.dt.bfloat16

EXP = mybir.ActivationFunctionType.Exp

_CACHE: dict = {}


def _build_bass(reps: int = 1, loop_reps: int = 0, ablate: tuple = (), batch: int = BATCH):
    """reps>1 unrolls the whole compute; loop_reps>0 wraps it in a hardware
    For_i loop instead (timing-only variants: slope between two loop_reps
    builds isolates per-iteration HW time from the ~100ms dispatch floor)."""
    nc = bacc.Bacc("TRN2", target_bir_lowering=False, debug=False)

    keff = nc.dram_tensor("keff", [C, N], BF16, kind="ExternalInput")
    qeff = nc.dram_tensor("qeff", [C, NSH], BF16, kind="ExternalInput")
    vaug = nc.dram_tensor("vaug", [N, CP], BF16, kind="ExternalInput")
    out = nc.dram_tensor("out", [NSH, C], F32, kind="ExternalOutput")

    KQ = 8                     # m-chunks per keff DMA tile
    KQN = MCN // KQ            # 4 keff tiles per c-chunk
    NB = MCN // batch          # QK/AV batches per n-tile

    with tile.TileContext(nc) as tc:
        with (
            tc.tile_pool(name="const", bufs=1) as cpool,
            tc.tile_pool(name="work", bufs=2 * batch) as wpool,
            tc.tile_pool(name="tail", bufs=2) as tpool,
            tc.tile_pool(name="dots_ps", bufs=batch, space="PSUM") as dots_pool,
            tc.tile_pool(name="acc_ps", bufs=1, space="PSUM") as acc_pool,
        ):
            zero = cpool.tile([P, 1], F32, tag="zero", name="zero")
            nc.vector.memset(zero[:], 0.0)

            qeff_t = []
            for cc in range(CCN):
                t = cpool.tile([P, NSH], BF16, tag=f"qeff{cc}", name=f"qeff{cc}")
                nc.sync.dma_start(t[:], qeff[cc * P : (cc + 1) * P, :])
                qeff_t.append(t)

            # keff split into [128, KQ*128] tiles so QK can start early
            keff_t = [[None] * KQN for _ in range(CCN)]
            for q in range(KQN):
                for cc in range(CCN):
                    t = cpool.tile([P, KQ * P], BF16, tag=f"keff{cc}_{q}", name=f"keff{cc}_{q}")
                    nc.sync.dma_start(
                        t[:], keff[cc * P : (cc + 1) * P, q * KQ * P : (q + 1) * KQ * P]
                    )
                    keff_t[cc][q] = t

            vaug_t = []
            for mc in range(MCN):
                t = cpool.tile([P, CP], BF16, tag=f"vaug{mc}", name=f"vaug{mc}")
                nc.sync.dma_start(t[:], vaug[mc * P : (mc + 1) * P, :])
                vaug_t.append(t)

            const_expt = None
            if "exp" in ablate:
                const_expt = cpool.tile([P, NT_SIZE], BF16, tag="cexpt", name="cexpt")
                nc.vector.memset(const_expt[:], 1.0)

            def emit_qk_batch(bi):
                """4 cc0 matmuls (one moving AP), then 4 cc1 matmuls."""
                nt, mcs = bi
                ns = slice(nt * NT_SIZE, (nt + 1) * NT_SIZE)
                dots = [
                    dots_pool.tile([P, NT_SIZE], F32, tag="dots", name="dots")
                    for _ in mcs
                ]
                for cc in range(CCN):
                    for x, mc in enumerate(mcs):
                        lhsT = keff_t[cc][mc // KQ][:, (mc % KQ) * P : (mc % KQ + 1) * P]
                        nc.tensor.matmul(
                            dots[x][:],
                            lhsT,
                            qeff_t[cc][:, ns],
                            start=(cc == 0),
                            stop=(cc == CCN - 1),
                        )
                return dots

            def emit_exp_batch(bi, dots):
                if "exp" in ablate:
                    return [const_expt] * len(dots)
                expts = []
                for d in dots:
                    e = wpool.tile([P, NT_SIZE], BF16, tag="expt", name="expt")
                    nc.scalar.activation(e[:], d[:], EXP, bias=zero[:], scale=SCALE)
                    expts.append(e)
                return expts

            def emit_body():
                def alloc_ut():
                    return [
                        acc_pool.tile([P, CP], F32, tag=f"ut{j}", name=f"ut{j}")
                        for j in range(JN)
                    ]

                ut = alloc_ut()

                def emit_tail(nt):
                    # outT rows: out[n,:] = utj[n, 0:C] * (1/Z[n]); Z = col C
                    for j in range(JN):
                        r0 = nt * NT_SIZE + j * P
                        recip = tpool.tile([P, 1], F32, tag="recip", name="recip")
                        nc.vector.reciprocal(recip[:], ut[j][:, C : C + 1])
                        outsb = tpool.tile([P, C], F32, tag="outsb", name="outsb")
                        nc.vector.tensor_scalar_mul(outsb[:], ut[j][:, 0:C], recip[:])
                        nc.sync.dma_start(out[r0 : r0 + P, :], outsb[:])

                def emit_av_batch(bi, expts):
                    nt, mcs = bi
                    for x, mc in enumerate(mcs):
                        first, last = mc == 0, mc == MCN - 1
                        for j in range(JN):
                            nc.tensor.matmul(
                                ut[j][:],
                                expts[x][:, j * P : (j + 1) * P],
                                vaug_t[mc][:],
                                start=first,
                                stop=last,
                            )

                batches = [
                    (nt, range(b * batch, (b + 1) * batch))
                    for nt in range(NTN)
                    for b in range(NB)
                ]
                noqk = "noqk" in ablate
                noav = "noav" in ablate

                def qk(bi):
                    if noqk:
                        return [
                            dots_pool.tile([P, NT_SIZE], F32, tag="dots", name="dots")
                            for _ in bi[1]
                        ]
                    return emit_qk_batch(bi)

                dots = qk(batches[0])
                for i, bi in enumerate(batches):
                    expts = emit_exp_batch(bi, dots)
                    if i + 1 < len(batches):
                        dots = qk(batches[i + 1])
                    if not noav:
                        emit_av_batch(bi, expts)
                    if bi[1][-1] == MCN - 1:
                        if "tail" not in ablate and not noav:
                            emit_tail(bi[0])
                        if bi[0] + 1 < NTN:
                            ut = alloc_ut()

            if loop_reps > 0:
                with tc.For_i(0, loop_reps, 1, hint_engines=(mybir.EngineType.PE,)):
                    emit_body()
            else:
                for _ in range(reps):
                    emit_body()

    nc.compile()
    return nc


def _prep_inputs(queries, keys, values, mask_eye, mask_mouth, q_pos, k_pos,
                 bias_eye, bias_mouth):
    """Host-side shard prep: positional adds, V transpose + ones column, and
    the per-batch scalar bias folded into V (v + c_b * colsum(V))."""
    q = queries.reshape(B, C, N) + q_pos[0].T[None]
    k = keys.reshape(B, C, N) + k_pos[0].T[None]
    vT = np.ascontiguousarray(values.reshape(B, C, N).transpose(0, 2, 1))  # [B,N,C]

    def msum(mask):
        # nearest resize 128->64 picks every other row/col
        m = mask[:, :, ::2, ::2].reshape(B, -1)
        return (m * m).sum(axis=1, dtype=np.float64)

    softplus = lambda x: np.logaddexp(0.0, x)
    c_b = softplus(float(bias_eye[0]) * msum(mask_eye)) + softplus(
        float(bias_mouth[0]) * msum(mask_mouth)
    )  # [B]
    S = vT.sum(axis=1, dtype=np.float64)  # [B, C]

    vaug = np.zeros((B, N, CP), ml_dtypes.bfloat16)
    vaug[:, :, :C] = (vT + (c_b[:, None] * S)[:, None, :]).astype(ml_dtypes.bfloat16)
    vaug[:, :, C] = 1.0

    kb = k.astype(ml_dtypes.bfloat16)
    qb = q.astype(ml_dtypes.bfloat16)

    in_maps = []
    for core in range(NCORES):
        b, sh = divmod(core, SHARDS_PER_B)
        n0 = sh * NSH
        in_maps.append(
            {
                "keff": np.ascontiguousarray(kb[b]),
                "qeff": np.ascontiguousarray(qb[b][:, n0 : n0 + NSH]),
                "vaug": vaug[b],
            }
        )
    return in_maps


def kernel(**inputs) -> np.ndarray:
    inputs = {k: np.asarray(v, np.float32) for k, v in inputs.items()}
    in_maps = _prep_inputs(**inputs)

    if "nc" not in _CACHE:
        _CACHE["nc"] = _build_bass()
    res = run_bass_kernel_spmd(_CACHE["nc"], in_maps, list(range(NCORES)))

    full = np.empty((B, C, N), np.float32)
    for core in range(NCORES):
        b, sh = divmod(core, SHARDS_PER_B)
        n0 = sh * NSH
        full[b][:, n0 : n0 + NSH] = res.results[core]["out"].T
    return full.reshape(B, C, H, W)
